# revision 42
# baseline (speedup 1.0000x reference)
"""Trainium2 Bass kernel for BCNet-style bilinear head.

Computes logits[b,h,n,d] = sum_k hm[h,k] * v_[b,n,k] * q_[b,d,k] + h_bias
where v_ = v @ wn(Wv,gv).T + bv,  q_ = q @ wn(Wq,gq).T + bq,
wn(W,g) = (g/||W||_F) * W.

Head-parallel M-route (120 GF total vs 150 GF for the GT-route):
expand the product; per head h (= per core):
  Mt[c',c]   = sum_k hm[h,k]*Wq'[k,c'] * Wv'[k,c]     (C x C, batch-indep)
  PT[c,bd]   = sum_c' Mt[c',c] * qT[c',bd] (+ u[c])   (u absorbs bq-term)
  out[b,n,d] = sum_c vT[b,c,n] * PT[c,b*D+d] + t3[b,d]
  t3[bd]     = sum_c' w[c'] * qT[c',bd] + t4          (bv-term + const)
with u[c] = sum_k hm*bq*Wv'[k,c], w[c'] = sum_k hm*bv*Wq'[k,c'],
t4 = sum_k hm*bv*bq + h_bias[h].
Sharding: head-parallel over H=8 across 8 cores; each core consumes the
full v/q (replicated) and emits out[:, h] — no collectives.
All matmuls bf16 with fp32 PSUM accumulation.

PSUM: one pool, 8 tags of [128,512] (16KB/part), reused by tag across
M / t3 / P / out phases. vT tiles ride a 48-slot ring over the retired
wqh/wv SBUF slots (M pass 2 walks kt in reverse so high-kt slots retire
first). P is software-pipelined one batch ahead of out to hide the
PSUM->SBUF copy latency.
"""

import sys

for _p in ("/opt/trn_rl_repo",):
    if _p not in sys.path:
        sys.path.insert(0, _p)

import numpy as np
import ml_dtypes

from concourse import bass, bacc, tile, mybir
from concourse.bass_utils import run_bass_kernel_spmd

BF16 = ml_dtypes.bfloat16
F32 = mybir.dt.float32
BF = mybir.dt.bfloat16
AF = mybir.ActivationFunctionType

B, N, C, D, K, H = 16, 1024, 1024, 128, 3072, 8
KT, CT, NT = K // 128, C // 128, N // 128  # 24, 8, 8
BD = B * D  # 2048
NCORES = 8
XU, XW, XT4 = 0, CT, 2 * CT  # cst columns: u tiles, w tiles, t4

_CACHE = {}


def _build_program(repeat=1):
    nc = bacc.Bacc("TRN2", target_bir_lowering=False, debug=False,
                   num_devices=NCORES)

    # wqh[kt,p,c'] = hm[h,k]*Wq'[k,c'], k = kt*128+p   (per-core, head h)
    wqh_d = nc.dram_tensor("wqh", [KT, 128, C], BF, kind="ExternalInput")
    wv_d = nc.dram_tensor("wv", [KT, 128, C], BF, kind="ExternalInput")
    # qT[ct,p,b*128+d] = q[b,d,ct*128+p]
    qT_d = nc.dram_tensor("qT", [CT, 128, BD], BF, kind="ExternalInput")
    # vT[b,ct,p,n] = v[b,n,ct*128+p]
    vT_d = nc.dram_tensor("vT", [B, CT, 128, N], BF, kind="ExternalInput")
    cst_d = nc.dram_tensor("cst", [128, 2 * CT + 1], F32, kind="ExternalInput")
    one_d = nc.dram_tensor("one", [128, 1], BF, kind="ExternalInput")
    oner_d = nc.dram_tensor("oner", [1, 128], BF, kind="ExternalInput")
    out_d = nc.dram_tensor("out", [B, N, D], BF, kind="ExternalOutput")

    with tile.TileContext(nc) as tc:
        with (
            tc.tile_pool(name="wq", bufs=1) as p_wq,
            tc.tile_pool(name="wv", bufs=1) as p_wv,
            tc.tile_pool(name="qt", bufs=1) as p_qt,
            tc.tile_pool(name="mt", bufs=1) as p_mt,
            tc.tile_pool(name="pt", bufs=1) as p_pt,
            tc.tile_pool(name="t3", bufs=1) as p_t3,
            tc.tile_pool(name="small", bufs=1) as p_small,
            tc.tile_pool(name="ob", bufs=1) as p_ob,
            tc.tile_pool(name="vt", bufs=1) as p_vt,
            tc.tile_pool(name="ps", bufs=1, space="PSUM") as ps,
        ):
          for rep in range(repeat):
            R = f"r{rep}_"
            # ---- DMA: kt=0 weights first so M starts immediately ----
            # M pass 1 consumes wq[kt] + the low c-half of wv[kt] at
            # 1.7us/kt; streaming only those (1.09us/kt) keeps the PE fed.
            # wv high halves follow afterward (pass 2's window). kt=0's wq
            # is further split so the first matmul waits on a 128KB DMA.
            wq_sb, wv_sb = [], []
            for kt in range(KT):
                tv = [p_wv.tile([128, 512], BF, tag=f"wv{kt}{hb}",
                                name=f"{R}wv{kt}{hb}") for hb in "ab"]
                wv_sb.append(tv)
                if kt == 0:
                    tq = tuple(
                        p_wq.tile([128, 512], BF, tag=f"wq0{hb}",
                                  name=f"{R}wq0{hb}") for hb in "ab")
                    nc.sync.dma_start(tq[0][:], wqh_d.ap()[0, :, 0:512])
                    nc.scalar.dma_start(tv[0][:], wv_d.ap()[0, :, 0:512])
                    nc.sync.dma_start(tq[1][:], wqh_d.ap()[0, :, 512:1024])
                    wq_sb.append(tq)
                else:
                    tq = p_wq.tile([128, C], BF, tag=f"wq{kt}",
                                   name=f"{R}wq{kt}")
                    nc.sync.dma_start(tq[:], wqh_d.ap()[kt])
                    wq_sb.append(tq)
                    nc.sync.dma_start(tv[0][:], wv_d.ap()[kt, :, 0:512])
                if kt == 0:
                    # small consts ride the scalar engine's DGE queue
                    cst_sb = p_small.tile([128, 2 * CT + 1], F32, tag="cst",
                                          name=f"{R}cst")
                    nc.scalar.dma_start(cst_sb[:], cst_d.ap())
                    one_sb = p_small.tile([128, 1], BF, tag="one",
                                          name=f"{R}one")
                    nc.scalar.dma_start(one_sb[:], one_d.ap())
                    oner_sb = p_small.tile([1, 128], BF, tag="oner",
                                           name=f"{R}oner")
                    nc.scalar.dma_start(oner_sb[:], oner_d.ap())
                    qt_sb = p_qt.tile([128, CT * BD], BF, tag="qt",
                                      name=f"{R}qt")


            # wv high halves (pass 2 runs kt reversed, so send them
            # reversed), then qT, then vT — all riding the bus window left
            # idle once the pass-1 stream ends
            for kt in range(KT - 1, -1, -1):
                nc.sync.dma_start(wv_sb[kt][1][:],
                                  wv_d.ap()[kt, :, 512:1024])
            for g in range(CT):
                nc.sync.dma_start(qt_sb[:, g * BD:(g + 1) * BD], qT_d.ap()[g])

            # ---- t3 partials on DVE (runs during M) ----
            ta = p_t3.tile([128, BD], BF, tag="ta", name=f"{R}ta")
            tb = p_t3.tile([128, BD], BF, tag="tb", name=f"{R}tb")
            nc.vector.tensor_scalar_mul(ta[:], qt_sb[:, 0:BD],
                                        cst_sb[:, XW:XW + 1])
            for ct in range(1, CT):
                nc.vector.tensor_scalar_mul(
                    tb[:], qt_sb[:, ct * BD:(ct + 1) * BD],
                    cst_sb[:, XW + ct:XW + ct + 1])
                nc.vector.tensor_tensor(ta[:], ta[:], tb[:],
                                        mybir.AluOpType.add)

            # ---- M: Mt[c',c] = sum_k wqh[k,c']*wv[k,c] ----
            # two c-half passes; pass 2 reversed so high-kt tiles retire
            # first (their SBUF slots become the vT ring, below)
            mt_sb = [p_mt.tile([128, C], BF, tag=f"mt{i}", name=f"{R}mt{i}")
                     for i in range(CT)]
            for half in range(2):
                kts = list(range(KT)) if half == 0 else \
                    list(range(KT - 1, -1, -1))
                pms = [ps.tile([128, 512], F32, tag=f"t{i}",
                               name=f"{R}pm{half}_{i}") for i in range(CT)]
                for kt in kts:
                    for i in range(CT):
                        if kt == 0:
                            lhsT = wq_sb[0][i // 4][:, (i % 4) * 128:
                                                    (i % 4 + 1) * 128]
                        else:
                            lhsT = wq_sb[kt][:, i * 128:(i + 1) * 128]
                        nc.tensor.matmul(
                            pms[i][:], lhsT, wv_sb[kt][half][:],
                            start=(kt == kts[0]), stop=(kt == kts[-1]))
                for i in range(CT):
                    dst = mt_sb[i][:, half * 512:(half + 1) * 512]
                    if i % 2 == 0:
                        nc.scalar.activation(dst, pms[i][:], AF.Copy)
                    else:
                        nc.vector.tensor_copy(dst, pms[i][:])

            # ---- t3 row: partition-reduce + t4. Issued on PE after P_0
            # (t3row is first needed by out_0's adds, which read it with a
            # partition-broadcast AP), so P_0 starts the moment M finishes.
            t3row = p_t3.tile([1, BD], BF, tag="t3row", name=f"{R}t3row")
            t3bc = p_t3.tile([128, BD], BF, tag="t3bc", name=f"{R}t3bc")

            def t3_phase():
                for j in range(4):
                    pt3 = ps.tile([1, 512], F32, tag=f"t{j}",
                                  name=f"{R}t3ps{j}")
                    nc.tensor.matmul(pt3[:], one_sb[:, 0:1],
                                     ta[:, j * 512:(j + 1) * 512],
                                     start=True, stop=True)
                    nc.scalar.activation(t3row[0:1, j * 512:(j + 1) * 512],
                                         pt3[:], AF.Identity,
                                         bias=cst_sb[0:1, XT4:XT4 + 1],
                                         scale=1.0)
                for j in range(4):
                    pb = ps.tile([128, 512], F32, tag=f"t{4 + j}",
                                 name=f"{R}t3bc{j}")
                    nc.tensor.matmul(pb[:], oner_sb[:],
                                     t3row[0:1, j * 512:(j + 1) * 512],
                                     start=True, stop=True)
                    if j % 2 == 0:
                        nc.scalar.activation(t3bc[:, j * 512:(j + 1) * 512],
                                             pb[:], AF.Copy)
                    else:
                        nc.vector.tensor_copy(
                            t3bc[:, j * 512:(j + 1) * 512], pb[:])

            # ---- per batch: P_b (pipelined one ahead) + out_{b-1} ----
            pt_sb = [p_pt.tile([128, BD], BF, tag=f"pt{i}", name=f"{R}pt{i}")
                     for i in range(CT)]
            vts = {}

            # vT ring: 7 dedicated fresh slots (usable before M retires
            # anything), the retired t3 scratch tiles (tb frees ~30us, ta
            # after the t3 reduce), then the 23 wq slots in pass-2 retire
            # order
            NVP = 5
            ring_slots = ([(p_vt, f"vtp{i}") for i in range(NVP)]
                          + [(p_t3, "tb"), (p_t3, "ta")]
                          + [(p_wq, f"wq{KT - 1 - i}") for i in range(KT - 1)])
            RING = len(ring_slots)

            def load_vt(b):
                vts[b] = []
                for ct in range(CT):
                    pool, tag = ring_slots[(b * CT + ct) % RING]
                    t = pool.tile([128, C], BF, tag=tag, name=f"{R}vt{b}_{ct}")
                    nc.sync.dma_start(t[:], vT_d.ap()[b, ct])
                    vts[b].append(t)

            def p_phase(b):
                for ct in range(CT):
                    pp = ps.tile([128, 128], F32, tag=f"t{ct}",
                                 name=f"{R}pp{b}_{ct}")
                    for j in range(CT):
                        nc.tensor.matmul(
                            pp[:],
                            mt_sb[j][:, ct * 128:(ct + 1) * 128],
                            qt_sb[:, j * BD + b * 128:j * BD + (b + 1) * 128],
                            start=(j == 0), stop=(j == CT - 1))
                    nc.scalar.activation(
                        pt_sb[ct][:, b * 128:(b + 1) * 128], pp[:],
                        AF.Identity, bias=cst_sb[:, XU + ct:XU + ct + 1],
                        scale=1.0)

            def out_phase(b):
                ob = p_ob.tile([128, NT * D], BF, tag=f"ob{b % 2}",
                               name=f"{R}ob{b}")
                for nt in range(NT):
                    po = ps.tile([128, 128], F32, tag=f"t{nt}",
                                 name=f"{R}po{b}_{nt}")
                    for ct in range(CT):
                        nc.tensor.matmul(
                            po[:],
                            vts[b][ct][:, nt * 128:(nt + 1) * 128],
                            pt_sb[ct][:, b * 128:(b + 1) * 128],
                            start=(ct == 0), stop=(ct == CT - 1))
                    nc.vector.tensor_tensor(
                        ob[:, nt * D:(nt + 1) * D], po[:],
                        t3bc[:, b * 128:(b + 1) * 128],
                        mybir.AluOpType.add)
                # one store per batch, on Activation's DGE queue (doesn't
                # block the sync-queue vT load stream); the final batch
                # stores per-nt on alternating queues so each fires right
                # after its own add (sub-range deps) and the tail shrinks
                if b == B - 1:
                    for nt in range(NT):
                        eng = nc.scalar if nt % 2 == 0 else nc.sync
                        eng.dma_start(
                            out_d.ap()[b, nt * 128:(nt + 1) * 128, :],
                            ob[:, nt * D:(nt + 1) * D])
                else:
                    nc.scalar.dma_start(
                        out_d.ap()[b].rearrange("(nt p) d -> p nt d", p=128),
                        ob[:].rearrange("p (nt d) -> p nt d", nt=NT))
                del vts[b]

            load_vt(0)
            p_phase(0)
            t3_phase()
            for b in range(1, B):
                load_vt(b)
                p_phase(b)
                out_phase(b - 1)
            out_phase(B - 1)

    nc.compile()
    return nc


def _get_program(repeat=1):
    key = f"nc{repeat}"
    if key not in _CACHE:
        _CACHE[key] = _build_program(repeat)
    return _CACHE[key]


def _prep_inputs(v, q, Wv, gv, bv, Wq, gq, bq, h_mat, h_bias):
    v = np.asarray(v, np.float32)
    q = np.asarray(q, np.float32)
    Wv = np.asarray(Wv, np.float32)
    Wq = np.asarray(Wq, np.float32)
    bv = np.asarray(bv, np.float32)
    bq = np.asarray(bq, np.float32)
    sv = np.float32(gv) / np.float32(np.linalg.norm(Wv))
    sq = np.float32(gq) / np.float32(np.linalg.norm(Wq))
    hm = np.asarray(h_mat, np.float32)[0, :, 0, :]  # (H, K)
    hb = np.asarray(h_bias, np.float32).reshape(H)

    Wvp = Wv * sv  # (K, C)
    Wqp = Wq * sq
    wv_b = np.ascontiguousarray(Wvp.reshape(KT, 128, C)).astype(BF16)
    qT = np.ascontiguousarray(
        q.transpose(2, 0, 1).reshape(CT, 128, BD)).astype(BF16)
    vT = np.ascontiguousarray(
        v.transpose(0, 2, 1).reshape(B, CT, 128, N)).astype(BF16)
    one = np.ones((128, 1), BF16)
    oner = np.ones((1, 128), BF16)

    in_maps = []
    for h in range(NCORES):
        wqh = np.ascontiguousarray(
            (hm[h][:, None] * Wqp).reshape(KT, 128, C)).astype(BF16)
        u = (hm[h] * bq) @ Wvp  # (C,)
        w = (hm[h] * bv) @ Wqp  # (C,)
        t4 = float((hm[h] * bv) @ bq) + float(hb[h])
        cst = np.zeros((128, 2 * CT + 1), np.float32)
        cst[:, XU:XU + CT] = u.reshape(CT, 128).T
        cst[:, XW:XW + CT] = w.reshape(CT, 128).T
        cst[0, XT4] = t4
        in_maps.append({
            "wqh": wqh,
            "wv": wv_b,
            "qT": qT,
            "vT": vT,
            "cst": cst,
            "one": one,
            "oner": oner,
        })
    return in_maps


def run_device(in_maps, **kw):
    nc = _get_program()
    return run_bass_kernel_spmd(nc, in_maps, list(range(NCORES)), **kw)


def kernel(v, q, Wv, gv, bv, Wq, gq, bq, h_mat, h_bias):
    in_maps = _prep_inputs(v, q, Wv, gv, bv, Wq, gq, bq, h_mat, h_bias)
    res = run_device(in_maps)
    out = np.empty((B, H, N, D), np.float32)
    for h in range(NCORES):
        out[:, h] = res.results[h]["out"].astype(np.float32)
    return out


if __name__ == "__main__":
    rng = np.random.default_rng(0)
    ins = {
        "v": rng.standard_normal((B, N, C), np.float32),
        "q": rng.standard_normal((B, D, C), np.float32),
        "Wv": rng.standard_normal((K, C), np.float32) * 0.02,
        "gv": np.ones((), np.float32),
        "bv": rng.standard_normal((K,), np.float32) * 0.02,
        "Wq": rng.standard_normal((K, C), np.float32) * 0.02,
        "gq": np.ones((), np.float32),
        "bq": rng.standard_normal((K,), np.float32) * 0.02,
        "h_mat": rng.standard_normal((1, H, 1, K), np.float32) * 0.02,
        "h_bias": np.zeros((1, H, 1, 1), np.float32),
    }
    out = kernel(**ins)
    print("out", out.shape, out.dtype, np.abs(out).max())


# revision 43
# speedup vs baseline: 1.0106x; 1.0106x over previous
"""Trainium2 Bass kernel for BCNet-style bilinear head.

Computes logits[b,h,n,d] = sum_k hm[h,k] * v_[b,n,k] * q_[b,d,k] + h_bias
where v_ = v @ wn(Wv,gv).T + bv,  q_ = q @ wn(Wq,gq).T + bq,
wn(W,g) = (g/||W||_F) * W.

Head-parallel M-route (120 GF total vs 150 GF for the GT-route):
expand the product; per head h (= per core):
  Mt[c',c]   = sum_k hm[h,k]*Wq'[k,c'] * Wv'[k,c]     (C x C, batch-indep)
  PT[c,bd]   = sum_c' Mt[c',c] * qT[c',bd] (+ u[c])   (u absorbs bq-term)
  out[b,n,d] = sum_c vT[b,c,n] * PT[c,b*D+d] + t3[b,d]
  t3[bd]     = sum_c' w[c'] * qT[c',bd] + t4          (bv-term + const)
with u[c] = sum_k hm*bq*Wv'[k,c], w[c'] = sum_k hm*bv*Wq'[k,c'],
t4 = sum_k hm*bv*bq + h_bias[h].
Sharding: head-parallel over H=8 across 8 cores; each core consumes the
full v/q (replicated) and emits out[:, h] — no collectives.
All matmuls bf16 with fp32 PSUM accumulation.

PSUM: one pool, 8 tags of [128,512] (16KB/part), reused by tag across
M / t3 / P / out phases. vT tiles ride a 48-slot ring over the retired
wqh/wv SBUF slots (M pass 2 walks kt in reverse so high-kt slots retire
first). P is software-pipelined one batch ahead of out to hide the
PSUM->SBUF copy latency.
"""

import sys

for _p in ("/opt/trn_rl_repo",):
    if _p not in sys.path:
        sys.path.insert(0, _p)

import numpy as np
import ml_dtypes

from concourse import bass, bacc, tile, mybir
from concourse.bass_utils import run_bass_kernel_spmd

BF16 = ml_dtypes.bfloat16
F32 = mybir.dt.float32
BF = mybir.dt.bfloat16
AF = mybir.ActivationFunctionType

B, N, C, D, K, H = 16, 1024, 1024, 128, 3072, 8
KT, CT, NT = K // 128, C // 128, N // 128  # 24, 8, 8
BD = B * D  # 2048
NCORES = 8
XU, XW, XT4 = 0, CT, 2 * CT  # cst columns: u tiles, w tiles, t4

_CACHE = {}


def _build_program(repeat=1):
    nc = bacc.Bacc("TRN2", target_bir_lowering=False, debug=False,
                   num_devices=NCORES)

    # wqh[kt,p,c'] = hm[h,k]*Wq'[k,c'], k = kt*128+p   (per-core, head h)
    wqh_d = nc.dram_tensor("wqh", [KT, 128, C], BF, kind="ExternalInput")
    wv_d = nc.dram_tensor("wv", [KT, 128, C], BF, kind="ExternalInput")
    # qT[ct,p,b*128+d] = q[b,d,ct*128+p]
    qT_d = nc.dram_tensor("qT", [CT, 128, BD], BF, kind="ExternalInput")
    # vT[b,ct,p,n] = v[b,n,ct*128+p]
    vT_d = nc.dram_tensor("vT", [B, CT, 128, N], BF, kind="ExternalInput")
    cst_d = nc.dram_tensor("cst", [128, 2 * CT + 1], F32, kind="ExternalInput")
    one_d = nc.dram_tensor("one", [128, 1], BF, kind="ExternalInput")
    oner_d = nc.dram_tensor("oner", [1, 128], BF, kind="ExternalInput")
    out_d = nc.dram_tensor("out", [B, N, D], BF, kind="ExternalOutput")

    with tile.TileContext(nc) as tc:
        with (
            tc.tile_pool(name="wq", bufs=1) as p_wq,
            tc.tile_pool(name="wv", bufs=1) as p_wv,
            tc.tile_pool(name="qt", bufs=1) as p_qt,
            tc.tile_pool(name="mt", bufs=1) as p_mt,
            tc.tile_pool(name="pt", bufs=1) as p_pt,
            tc.tile_pool(name="t3", bufs=1) as p_t3,
            tc.tile_pool(name="small", bufs=1) as p_small,
            tc.tile_pool(name="ob", bufs=1) as p_ob,
            tc.tile_pool(name="vt", bufs=1) as p_vt,
            tc.tile_pool(name="ps", bufs=1, space="PSUM") as ps,
        ):
          for rep in range(repeat):
            R = f"r{rep}_"
            # ---- DMA: kt=0 weights first so M starts immediately ----
            # M pass 1 consumes wq[kt] + the low c-half of wv[kt] at
            # 1.7us/kt; streaming only those (1.09us/kt) keeps the PE fed.
            # wv high halves follow afterward (pass 2's window). kt=0's wq
            # is further split so the first matmul waits on a 128KB DMA.
            wq_sb, wv_sb = [], []
            for kt in range(KT):
                tv = [p_wv.tile([128, 512], BF, tag=f"wv{kt}{hb}",
                                name=f"{R}wv{kt}{hb}") for hb in "ab"]
                wv_sb.append(tv)
                if kt == 0:
                    tq = tuple(
                        p_wq.tile([128, 512], BF, tag=f"wq0{hb}",
                                  name=f"{R}wq0{hb}") for hb in "ab")
                    nc.sync.dma_start(tq[0][:], wqh_d.ap()[0, :, 0:512])
                    nc.scalar.dma_start(tv[0][:], wv_d.ap()[0, :, 0:512])
                    nc.scalar.dma_start(tq[1][:], wqh_d.ap()[0, :, 512:1024])
                    wq_sb.append(tq)
                else:
                    tq = p_wq.tile([128, C], BF, tag=f"wq{kt}",
                                   name=f"{R}wq{kt}")
                    nc.sync.dma_start(tq[:], wqh_d.ap()[kt])
                    wq_sb.append(tq)
                    nc.sync.dma_start(tv[0][:], wv_d.ap()[kt, :, 0:512])
                if kt == 0:
                    # small consts ride the scalar engine's DGE queue
                    cst_sb = p_small.tile([128, 2 * CT + 1], F32, tag="cst",
                                          name=f"{R}cst")
                    nc.scalar.dma_start(cst_sb[:], cst_d.ap())
                    one_sb = p_small.tile([128, 1], BF, tag="one",
                                          name=f"{R}one")
                    nc.scalar.dma_start(one_sb[:], one_d.ap())
                    oner_sb = p_small.tile([1, 128], BF, tag="oner",
                                           name=f"{R}oner")
                    nc.scalar.dma_start(oner_sb[:], oner_d.ap())
                    qt_sb = p_qt.tile([128, CT * BD], BF, tag="qt",
                                      name=f"{R}qt")


            # wv high halves (pass 2 runs kt reversed, so send them
            # reversed), then qT, then vT — all riding the bus window left
            # idle once the pass-1 stream ends
            for kt in range(KT - 1, -1, -1):
                nc.sync.dma_start(wv_sb[kt][1][:],
                                  wv_d.ap()[kt, :, 512:1024])
            for g in range(CT):
                nc.sync.dma_start(qt_sb[:, g * BD:(g + 1) * BD], qT_d.ap()[g])

            # ---- t3 partials on DVE (runs during M) ----
            ta = p_t3.tile([128, BD], BF, tag="ta", name=f"{R}ta")
            tb = p_t3.tile([128, BD], BF, tag="tb", name=f"{R}tb")
            nc.vector.tensor_scalar_mul(ta[:], qt_sb[:, 0:BD],
                                        cst_sb[:, XW:XW + 1])
            for ct in range(1, CT):
                nc.vector.tensor_scalar_mul(
                    tb[:], qt_sb[:, ct * BD:(ct + 1) * BD],
                    cst_sb[:, XW + ct:XW + ct + 1])
                nc.vector.tensor_tensor(ta[:], ta[:], tb[:],
                                        mybir.AluOpType.add)

            # ---- M: Mt[c',c] = sum_k wqh[k,c']*wv[k,c] ----
            # two c-half passes; pass 2 reversed so high-kt tiles retire
            # first (their SBUF slots become the vT ring, below)
            mt_sb = [p_mt.tile([128, C], BF, tag=f"mt{i}", name=f"{R}mt{i}")
                     for i in range(CT)]
            for half in range(2):
                kts = list(range(KT)) if half == 0 else \
                    list(range(KT - 1, -1, -1))
                pms = [ps.tile([128, 512], F32, tag=f"t{i}",
                               name=f"{R}pm{half}_{i}") for i in range(CT)]
                for kt in kts:
                    for i in range(CT):
                        if kt == 0:
                            lhsT = wq_sb[0][i // 4][:, (i % 4) * 128:
                                                    (i % 4 + 1) * 128]
                        else:
                            lhsT = wq_sb[kt][:, i * 128:(i + 1) * 128]
                        nc.tensor.matmul(
                            pms[i][:], lhsT, wv_sb[kt][half][:],
                            start=(kt == kts[0]), stop=(kt == kts[-1]))
                for i in range(CT):
                    dst = mt_sb[i][:, half * 512:(half + 1) * 512]
                    if i % 2 == 0:
                        nc.scalar.activation(dst, pms[i][:], AF.Copy)
                    else:
                        nc.vector.tensor_copy(dst, pms[i][:])

            # ---- t3 row: partition-reduce + t4. Issued on PE after P_0
            # (t3row is first needed by out_0's adds, which read it with a
            # partition-broadcast AP), so P_0 starts the moment M finishes.
            t3row = p_t3.tile([1, BD], BF, tag="t3row", name=f"{R}t3row")
            t3bc = p_t3.tile([128, BD], BF, tag="t3bc", name=f"{R}t3bc")

            def t3_phase():
                for j in range(4):
                    pt3 = ps.tile([1, 512], F32, tag=f"t{j}",
                                  name=f"{R}t3ps{j}")
                    nc.tensor.matmul(pt3[:], one_sb[:, 0:1],
                                     ta[:, j * 512:(j + 1) * 512],
                                     start=True, stop=True)
                    nc.scalar.activation(t3row[0:1, j * 512:(j + 1) * 512],
                                         pt3[:], AF.Identity,
                                         bias=cst_sb[0:1, XT4:XT4 + 1],
                                         scale=1.0)
                for j in range(4):
                    pb = ps.tile([128, 512], F32, tag=f"t{4 + j}",
                                 name=f"{R}t3bc{j}")
                    nc.tensor.matmul(pb[:], oner_sb[:],
                                     t3row[0:1, j * 512:(j + 1) * 512],
                                     start=True, stop=True)
                    if j % 2 == 0:
                        nc.scalar.activation(t3bc[:, j * 512:(j + 1) * 512],
                                             pb[:], AF.Copy)
                    else:
                        nc.vector.tensor_copy(
                            t3bc[:, j * 512:(j + 1) * 512], pb[:])

            # ---- per batch: P_b (pipelined one ahead) + out_{b-1} ----
            pt_sb = [p_pt.tile([128, BD], BF, tag=f"pt{i}", name=f"{R}pt{i}")
                     for i in range(CT)]
            vts = {}

            # vT ring: 7 dedicated fresh slots (usable before M retires
            # anything), the retired t3 scratch tiles (tb frees ~30us, ta
            # after the t3 reduce), then the 23 wq slots in pass-2 retire
            # order
            NVP = 5
            ring_slots = ([(p_vt, f"vtp{i}") for i in range(NVP)]
                          + [(p_t3, "tb"), (p_t3, "ta")]
                          + [(p_wq, f"wq{KT - 1 - i}") for i in range(KT - 1)])
            RING = len(ring_slots)

            def load_vt(b):
                vts[b] = []
                for ct in range(CT):
                    pool, tag = ring_slots[(b * CT + ct) % RING]
                    t = pool.tile([128, C], BF, tag=tag, name=f"{R}vt{b}_{ct}")
                    nc.sync.dma_start(t[:], vT_d.ap()[b, ct])
                    vts[b].append(t)

            def p_phase(b):
                for ct in range(CT):
                    pp = ps.tile([128, 128], F32, tag=f"t{ct}",
                                 name=f"{R}pp{b}_{ct}")
                    for j in range(CT):
                        nc.tensor.matmul(
                            pp[:],
                            mt_sb[j][:, ct * 128:(ct + 1) * 128],
                            qt_sb[:, j * BD + b * 128:j * BD + (b + 1) * 128],
                            start=(j == 0), stop=(j == CT - 1))
                    nc.scalar.activation(
                        pt_sb[ct][:, b * 128:(b + 1) * 128], pp[:],
                        AF.Identity, bias=cst_sb[:, XU + ct:XU + ct + 1],
                        scale=1.0)

            def out_phase(b):
                ob = p_ob.tile([128, NT * D], BF, tag=f"ob{b % 2}",
                               name=f"{R}ob{b}")
                for nt in range(NT):
                    po = ps.tile([128, 128], F32, tag=f"t{nt}",
                                 name=f"{R}po{b}_{nt}")
                    for ct in range(CT):
                        nc.tensor.matmul(
                            po[:],
                            vts[b][ct][:, nt * 128:(nt + 1) * 128],
                            pt_sb[ct][:, b * 128:(b + 1) * 128],
                            start=(ct == 0), stop=(ct == CT - 1))
                    nc.vector.tensor_tensor(
                        ob[:, nt * D:(nt + 1) * D], po[:],
                        t3bc[:, b * 128:(b + 1) * 128],
                        mybir.AluOpType.add)
                # one store per batch, on Activation's DGE queue (doesn't
                # block the sync-queue vT load stream); the final batch
                # stores per-nt on alternating queues so each fires right
                # after its own add (sub-range deps) and the tail shrinks
                if b == B - 1:
                    for g in range(4):
                        eng = nc.scalar if g % 2 == 0 else nc.sync
                        eng.dma_start(
                            out_d.ap()[b, g * 256:(g + 1) * 256, :]
                            .rearrange("(nt p) d -> p nt d", p=128),
                            ob[:, g * 2 * D:(g + 1) * 2 * D]
                            .rearrange("p (nt d) -> p nt d", nt=2))
                else:
                    nc.scalar.dma_start(
                        out_d.ap()[b].rearrange("(nt p) d -> p nt d", p=128),
                        ob[:].rearrange("p (nt d) -> p nt d", nt=NT))
                del vts[b]

            load_vt(0)
            p_phase(0)
            t3_phase()
            for b in range(1, B):
                load_vt(b)
                p_phase(b)
                out_phase(b - 1)
            out_phase(B - 1)

    nc.compile()
    return nc


def _get_program(repeat=1):
    key = f"nc{repeat}"
    if key not in _CACHE:
        _CACHE[key] = _build_program(repeat)
    return _CACHE[key]


def _prep_inputs(v, q, Wv, gv, bv, Wq, gq, bq, h_mat, h_bias):
    v = np.asarray(v, np.float32)
    q = np.asarray(q, np.float32)
    Wv = np.asarray(Wv, np.float32)
    Wq = np.asarray(Wq, np.float32)
    bv = np.asarray(bv, np.float32)
    bq = np.asarray(bq, np.float32)
    sv = np.float32(gv) / np.float32(np.linalg.norm(Wv))
    sq = np.float32(gq) / np.float32(np.linalg.norm(Wq))
    hm = np.asarray(h_mat, np.float32)[0, :, 0, :]  # (H, K)
    hb = np.asarray(h_bias, np.float32).reshape(H)

    Wvp = Wv * sv  # (K, C)
    Wqp = Wq * sq
    wv_b = np.ascontiguousarray(Wvp.reshape(KT, 128, C)).astype(BF16)
    qT = np.ascontiguousarray(
        q.transpose(2, 0, 1).reshape(CT, 128, BD)).astype(BF16)
    vT = np.ascontiguousarray(
        v.transpose(0, 2, 1).reshape(B, CT, 128, N)).astype(BF16)
    one = np.ones((128, 1), BF16)
    oner = np.ones((1, 128), BF16)

    in_maps = []
    for h in range(NCORES):
        wqh = np.ascontiguousarray(
            (hm[h][:, None] * Wqp).reshape(KT, 128, C)).astype(BF16)
        u = (hm[h] * bq) @ Wvp  # (C,)
        w = (hm[h] * bv) @ Wqp  # (C,)
        t4 = float((hm[h] * bv) @ bq) + float(hb[h])
        cst = np.zeros((128, 2 * CT + 1), np.float32)
        cst[:, XU:XU + CT] = u.reshape(CT, 128).T
        cst[:, XW:XW + CT] = w.reshape(CT, 128).T
        cst[0, XT4] = t4
        in_maps.append({
            "wqh": wqh,
            "wv": wv_b,
            "qT": qT,
            "vT": vT,
            "cst": cst,
            "one": one,
            "oner": oner,
        })
    return in_maps


def run_device(in_maps, **kw):
    nc = _get_program()
    return run_bass_kernel_spmd(nc, in_maps, list(range(NCORES)), **kw)


def kernel(v, q, Wv, gv, bv, Wq, gq, bq, h_mat, h_bias):
    in_maps = _prep_inputs(v, q, Wv, gv, bv, Wq, gq, bq, h_mat, h_bias)
    res = run_device(in_maps)
    out = np.empty((B, H, N, D), np.float32)
    for h in range(NCORES):
        out[:, h] = res.results[h]["out"].astype(np.float32)
    return out


if __name__ == "__main__":
    rng = np.random.default_rng(0)
    ins = {
        "v": rng.standard_normal((B, N, C), np.float32),
        "q": rng.standard_normal((B, D, C), np.float32),
        "Wv": rng.standard_normal((K, C), np.float32) * 0.02,
        "gv": np.ones((), np.float32),
        "bv": rng.standard_normal((K,), np.float32) * 0.02,
        "Wq": rng.standard_normal((K, C), np.float32) * 0.02,
        "gq": np.ones((), np.float32),
        "bq": rng.standard_normal((K,), np.float32) * 0.02,
        "h_mat": rng.standard_normal((1, H, 1, K), np.float32) * 0.02,
        "h_bias": np.zeros((1, H, 1, 1), np.float32),
    }
    out = kernel(**ins)
    print("out", out.shape, out.dtype, np.abs(out).max())
